# revision 40
# baseline (speedup 1.0000x reference)
"""Trainium2 Bass kernel for decode-style single-query MultiHeadAttention.

Reference computation (L=8192, E=1024, H=16, D=64):
    q = x[:1] @ Wq.T + bq                  # [1, E]
    k = x @ Wk.T + bk                      # [L, E]
    v = x @ Wv.T + bv                      # [L, E]
    per head: out_h = softmax(q_h k_h^T / sqrt(D)) v_h
    out = concat(out_h) @ Wo.T + bo        # [1, E]

Key algebraic factorization (exact, just reassociated):
    scores_h[l] = (q_h @ Wk_h) . x[l] * scale   (+ const per head -> softmax-invariant)
    attn_h @ V_h = (attn_h @ x) @ Wv_h.T + bv_h
so the device only ever contracts x against tiny [16 x E] operands
(~34 MFLOP/core) instead of materializing K/V (~4.3 GFLOP/core).

Sharding: x is split along L across the 8 cores (1024 rows each), and each
core splits its chunk into 2 flash blocks of 512 rows. Per block b:
    s_b = w @ x_b^T     [16, 512]   (w = scaled q-projected K-weights)
    m_b = rowmax(s_b), P_b = exp(s_b - m_b), d_b = rowsum(P_b)
    z_b = P_b @ x_b     [16, 1024]  (unnormalized attn @ x)
The host ships both xc and xc^T (so the device does zero transposes of x)
and does the tiny glue math: q/w preparation, flash-style softmax combine
across the 16 blocks, the V/out projections.

Env knobs:
    KERNEL_MM = bf16 (default) | f32r | f32   -- matmul operand dtype
    KERNEL_XT = host (default) | dev          -- x^T shipped from host or
                                                 built on device (PE+DVE)
"""

import os
import numpy as np
from contextlib import ExitStack

L, E, H, D = 8192, 1024, 16, 64
NCORES = 8
NL = L // NCORES  # 1024 rows of x per core
EJ = E // 128     # 8 e-chunks
LJ = NL // 128    # 8 l-chunks per core
NHALF = 2         # flash blocks per core
SCALE = 1.0 / np.sqrt(np.float32(D))

_PROG = None
_PROG_KEY = None
last_exec_time_ns = None
last_results = None

MM_MODE = os.environ.get("KERNEL_MM", "bf16")
XT_MODE = os.environ.get("KERNEL_XT", "host")


def _xdt(mybir):
    return {
        "f32": mybir.dt.float32,
        "f32r": mybir.dt.float32r,
        "bf16": mybir.dt.bfloat16,
    }[MM_MODE]


def to_dev_dtype(a):
    """Convert fp32 host array to the device matmul operand dtype."""
    a = np.ascontiguousarray(a, dtype=np.float32)
    if MM_MODE == "f32":
        return a
    if MM_MODE == "bf16":
        import ml_dtypes

        return np.ascontiguousarray(a.astype(ml_dtypes.bfloat16))
    # f32r = TF32: round to 10-bit mantissa (RNE) so host bits match HW rounding
    u = a.view(np.uint32)
    lsb = (u >> np.uint32(13)) & np.uint32(1)
    r = (u + np.uint32(0x0FFF) + lsb) & np.uint32(0xFFFFE000)
    return r.view(np.float32)


def _emit(tc, tens):
    from concourse import mybir

    nc = tc.nc
    f32 = mybir.dt.float32
    xdt = _xdt(mybir)

    with ExitStack() as ctx:
        sb = ctx.enter_context(tc.tile_pool(name="sb", bufs=1))
        pst = ctx.enter_context(tc.tile_pool(name="pst", bufs=2, space="PSUM"))
        pss = ctx.enter_context(tc.tile_pool(name="pss", bufs=1, space="PSUM"))
        psz = ctx.enter_context(tc.tile_pool(name="psz", bufs=1, space="PSUM"))

        wt_sb = sb.tile([128, EJ * H], xdt)
        nc.scalar.dma_start(wt_sb[:], tens["wt"][:])
        id16 = sb.tile([H, H], f32)
        nc.scalar.dma_start(id16[:], tens["id16"][:])

        # Prewarm the ACT Exp table so LoadActFuncSet happens during the DMA
        # phase instead of on the softmax critical path.
        warm = sb.tile([1, 1], f32)
        nc.gpsimd.memset(warm[:], 0.0)
        warm2 = sb.tile([1, 1], f32)
        nc.scalar.activation(warm2[:], warm[:], mybir.ActivationFunctionType.Exp)

        # xT e-chunk i ([128 e, NL l]) lives at xt_all[:, i*NL:(i+1)*NL]
        xt_all = sb.tile([128, EJ * NL], xdt)
        # x l-chunk j ([128 l, E]) lives at x_all[:, j*E:(j+1)*E]
        x_all = sb.tile([128, LJ * E], xdt)

        # scores PSUM: one tile per flash block so each block's softmax can
        # start the moment its own accumulation group finishes
        s_half = [
            pss.tile([H, 512], f32, tag=f"s{hb}", name="s_half") for hb in range(NHALF)
        ]

        if XT_MODE == "host":
            # Big DMAs in exact stream order on the SP ring: [xtA, xtB,
            # xcA0, xcA1, xcB0, xcB1]. Block A's softmax+z pipeline runs
            # while block B's bytes are still in flight; per-DMA issue cost
            # stays hidden behind the previous transfer.
            xtc_3d = tens["xtc"].rearrange("(i p) l -> p i l", p=128)
            xt_3d = xt_all.rearrange("p (i l) -> p i l", i=EJ)
            xc_3d = tens["xc"].rearrange("(a p) e -> p a e", p=128)
            xa_3d = x_all.rearrange("p (a e) -> p a e", a=LJ)
            for hb in range(NHALF):
                for i2 in range(2):
                    nc.sync.dma_start(
                        xt_3d[:, i2 * 4:(i2 + 1) * 4, hb * 512:(hb + 1) * 512],
                        xtc_3d[:, i2 * 4:(i2 + 1) * 4, hb * 512:(hb + 1) * 512],
                    )
            # last pair split in two so the final z matmuls start one chunk
            # earlier
            for j2 in range(LJ // 2 - 1):
                nc.sync.dma_start(
                    xa_3d[:, 2 * j2:2 * j2 + 2, :],
                    xc_3d[:, 2 * j2:2 * j2 + 2, :],
                )
            for j in (LJ - 2, LJ - 1):
                nc.sync.dma_start(
                    xa_3d[:, j:j + 1, :], xc_3d[:, j:j + 1, :]
                )
            # PE clock-ramp warmers: harmless matmuls so the PE clock (HAM,
            # ~3.4us activity window) is ramped before the real score matmuls
            # start. In bf16 mode warm against a memset tile so the warmers
            # have no DMA dependency at all; in f32r mode operands must come
            # from an f32r-rounding producer, so use the wt tile (lands via
            # the first tiny DMA).
            if MM_MODE == "bf16":
                wz = sb.tile([128, 128], xdt)
                nc.gpsimd.memset(wz[:], 0.0)
                warm_lhs, warm_rhs = wz[:, :H], wz[:, :128]
            else:
                warm_lhs, warm_rhs = wt_sb[:, :H], wt_sb[:, :128]
            for _ in range(10):
                nc.tensor.matmul(
                    s_half[0][:, :128], warm_lhs, warm_rhs,
                    start=True, stop=True,
                )
        else:
            id128 = sb.tile([128, 128], xdt)
            nc.sync.dma_start(id128[:], tens["id128"][:])
            for j in range(LJ):
                eng = nc.sync if j % 2 == 0 else nc.scalar
                eng.dma_start(
                    x_all[:, j * E:(j + 1) * E], tens["xc"][j * 128:(j + 1) * 128, :]
                )
            for j in range(LJ):
                for i in range(EJ):
                    tr = pst.tile([128, 128], xdt, tag="tr", name="tr")
                    nc.tensor.transpose(
                        tr[:], x_all[:, j * E + i * 128: j * E + (i + 1) * 128], id128[:]
                    )
                    nc.vector.tensor_copy(
                        xt_all[:, i * NL + j * 128: i * NL + (j + 1) * 128], tr[:]
                    )

        # scores: s[h, l] = sum_e w[h, e] * xc[l, e] (scale folded into w).
        # Flash block (hb) outer so block 0's scores finish first.
        for hb in range(NHALF):
            for i in range(EJ):
                nc.tensor.matmul(
                    s_half[hb][:],
                    wt_sb[:, i * H:(i + 1) * H],
                    xt_all[:, i * NL + hb * 512: i * NL + (hb + 1) * 512],
                    start=(i == 0),
                    stop=(i == EJ - 1),
                )

        # Softmax partials, stage-major across the two flash blocks so block
        # B's reduce/exp are not stuck behind block A's P^T copies in the
        # DVE/ACT FIFOs; the PE queue stays block-major (pt_a, z_a, pt_b,
        # z_b) so z_a never stalls behind a pt_b transpose that is still
        # waiting on exp_b.
        p_sb = sb.tile([H, NL], f32)
        pt_all = sb.tile([128, LJ * H], xdt)
        md_sb = sb.tile([H, 2 * NHALF], f32)
        z_sb = sb.tile([H, NHALF * E], f32)
        negm, dsum, z_ps = [], [], []
        for hb in range(NHALF):
            nm = sb.tile([H, 1], f32, tag=f"negm{hb}", name="negm")
            nc.vector.reduce_max(
                nm[:], s_half[hb][:], axis=mybir.AxisListType.X, negate=True
            )
            negm.append(nm)
        for hb in range(NHALF):
            ds = sb.tile([H, 1], f32, tag=f"dsum{hb}", name="dsum")
            nc.scalar.activation(
                p_sb[:, hb * 512:(hb + 1) * 512],
                s_half[hb][:],
                mybir.ActivationFunctionType.Exp,
                bias=negm[hb][:], scale=1.0, accum_out=ds[:],
            )
            dsum.append(ds)
        for hb in range(NHALF):
            zp = psz.tile([H, E], f32, tag=f"z{hb}", name="zps")
            z_ps.append(zp)
            for j in range(4 * hb, 4 * hb + 4):
                ptr = pst.tile([128, H], f32, tag="ptr", name="ptr")
                nc.tensor.transpose(ptr[:], p_sb[:, j * 128:(j + 1) * 128], id16[:])
                nc.vector.tensor_copy(pt_all[:, j * H:(j + 1) * H], ptr[:])
            for j in range(4 * hb, 4 * hb + 4):
                # on the last chunk do the zb=1 half first so its PSUM->SBUF
                # copy (on the other engine) overlaps the zb=0 matmul
                zbs = (1, 0) if j == 4 * hb + 3 else (0, 1)
                for zb in zbs:
                    nc.tensor.matmul(
                        zp[:, zb * 512:(zb + 1) * 512],
                        pt_all[:, j * H:(j + 1) * H],
                        x_all[:, j * E + zb * 512: j * E + (zb + 1) * 512],
                        start=(j == 4 * hb),
                        stop=(j == 4 * hb + 3),
                    )
        # md partials first (tiny, keeps them off the end-of-kernel path)
        for hb in range(NHALF):
            nc.vector.tensor_copy(md_sb[:, 2 * hb:2 * hb + 1], negm[hb][:])
            nc.vector.tensor_copy(md_sb[:, 2 * hb + 1:2 * hb + 2], dsum[hb][:])
        nc.scalar.dma_start(tens["mdout"][:], md_sb[:])

        # PSUM -> SBUF -> DRAM per block, one consolidated zout DMA each.
        # Block A's copies go to ACT (idle after the exps, and DVE's FIFO is
        # still full of P^T copies); block B's are split DVE/ACT so they run
        # in parallel right after the last z matmul.
        nc.scalar.copy(z_sb[:, 0:512], z_ps[0][:, 0:512])
        nc.scalar.copy(z_sb[:, 512:1024], z_ps[0][:, 512:1024])
        nc.sync.dma_start(tens["zout"][:, 0:E], z_sb[:, 0:E])
        nc.vector.tensor_copy(z_sb[:, E:E + 512], z_ps[1][:, 0:512])
        nc.scalar.copy(z_sb[:, E + 512:2 * E], z_ps[1][:, 512:1024])
        nc.sync.dma_start(tens["zout"][:, E:2 * E], z_sb[:, E:2 * E])


def _build_program():
    import concourse.tile as tile
    from concourse import bacc, mybir

    f32 = mybir.dt.float32
    xdt = _xdt(mybir)
    nc = bacc.Bacc("TRN2", target_bir_lowering=False, debug=False, num_devices=NCORES)
    tens = {
        "xc": nc.dram_tensor("xc", [NL, E], xdt, kind="ExternalInput").ap(),
        "wt": nc.dram_tensor("wt", [128, EJ * H], xdt, kind="ExternalInput").ap(),
        "id16": nc.dram_tensor("id16", [H, H], f32, kind="ExternalInput").ap(),
        "zout": nc.dram_tensor("zout", [H, NHALF * E], f32, kind="ExternalOutput").ap(),
        "mdout": nc.dram_tensor("mdout", [H, 2 * NHALF], f32, kind="ExternalOutput").ap(),
    }
    if XT_MODE == "host":
        tens["xtc"] = nc.dram_tensor("xtc", [E, NL], xdt, kind="ExternalInput").ap()
    else:
        tens["id128"] = nc.dram_tensor("id128", [128, 128], xdt, kind="ExternalInput").ap()

    with tile.TileContext(nc) as tc:
        _emit(tc, tens)
    nc.compile()
    return nc


def get_prog():
    global _PROG, _PROG_KEY
    key = (MM_MODE, XT_MODE)
    if _PROG is None or _PROG_KEY != key:
        _PROG = _build_program()
        _PROG_KEY = key
    return _PROG


def make_in_maps(x, in_proj_weight, in_proj_bias):
    """Host prep: q projection + scaled score weights, sharded x (+x^T) chunks."""
    xd = to_dev_dtype(x)  # [L, E] device dtype
    Wq = np.asarray(in_proj_weight[:E], dtype=np.float64)
    Wk = np.asarray(in_proj_weight[E:2 * E], dtype=np.float64)
    bq = np.asarray(in_proj_bias[:E], dtype=np.float64)

    q = np.asarray(x[0:1], dtype=np.float64) @ Wq.T + bq  # [1, E]
    qh = q.reshape(H, D)                                # [16, 64]
    Wkh = Wk.reshape(H, D, E)                           # [16, 64, 1024]
    w = float(SCALE) * np.einsum("hd,hde->he", qh, Wkh)  # [16, 1024]
    # device layout: wt[p, i*H + h] = w[h, i*128 + p]
    wt = to_dev_dtype(
        w.astype(np.float32).T.reshape(EJ, 128, H).transpose(1, 0, 2).reshape(128, EJ * H)
    )
    id16 = np.eye(H, dtype=np.float32)
    maps = []
    for c in range(NCORES):
        xc = np.ascontiguousarray(xd[c * NL:(c + 1) * NL])
        m = {"xc": xc, "wt": wt, "id16": id16}
        if XT_MODE == "host":
            m["xtc"] = np.ascontiguousarray(xc.T)
        else:
            m["id128"] = to_dev_dtype(np.eye(128, dtype=np.float32))
        maps.append(m)
    return maps


def combine(z, md, in_proj_weight, in_proj_bias, out_proj_weight, out_proj_bias):
    """Flash-style softmax combine across partial blocks + V / out projections.

    z:  [nblocks, H, E]  unnormalized P @ x per block
    md: [nblocks, H, 2]  (-max, expsum) per block
    """
    Wv = np.asarray(in_proj_weight[2 * E:], dtype=np.float64)
    bv = np.asarray(in_proj_bias[2 * E:], dtype=np.float64)

    m = -md[:, :, 0].astype(np.float64)                 # [nb, 16] per-block max
    d = md[:, :, 1].astype(np.float64)                  # [nb, 16] per-block expsum
    M = m.max(axis=0)                                   # [16]
    alpha = np.exp(m - M)                               # [nb, 16]
    Dn = (d * alpha).sum(axis=0)                        # [16]
    Z = (z.astype(np.float64) * alpha[:, :, None]).sum(axis=0) / Dn[:, None]  # [16, E]

    o = np.einsum("he,hde->hd", Z, Wv.reshape(H, D, E)) + bv.reshape(H, D)  # [16, 64]
    o = o.reshape(1, E)
    out = o @ np.asarray(out_proj_weight, dtype=np.float64).T + np.asarray(
        out_proj_bias, dtype=np.float64
    )
    return out.astype(np.float32)


def run_device(in_maps, trace=False):
    from concourse import bass_utils

    global last_exec_time_ns, last_results
    nc = get_prog()
    res = bass_utils.run_bass_kernel_spmd(
        nc, in_maps, core_ids=list(range(NCORES)), trace=trace
    )
    last_exec_time_ns = res.exec_time_ns
    last_results = res
    return res


def unpack_outputs(res):
    """Device outputs -> (z [nblocks, H, E], md [nblocks, H, 2])."""
    z, md = [], []
    for c in range(NCORES):
        zc = res.results[c]["zout"]    # [H, NHALF*E]
        mc = res.results[c]["mdout"]   # [H, 2*NHALF]
        for hb in range(NHALF):
            z.append(zc[:, hb * E:(hb + 1) * E])
            md.append(mc[:, 2 * hb:2 * hb + 2])
    return np.stack(z), np.stack(md)


def kernel(x, in_proj_weight, in_proj_bias, out_proj_weight, out_proj_bias):
    in_maps = make_in_maps(x, in_proj_weight, in_proj_bias)
    res = run_device(in_maps, trace=os.environ.get("KERNEL_TRACE", "") == "1")
    z, md = unpack_outputs(res)
    return combine(z, md, in_proj_weight, in_proj_bias, out_proj_weight, out_proj_bias)
